# revision 4
# baseline (speedup 1.0000x reference)
"""MoE layer (B=2,T=1024,D=2048,F=768,E=16,K=2) on 8 NeuronCores.

Expert-parallel: core c owns experts {2c, 2c+1}. Host computes the router
(scores -> softmax -> top-2 -> renormalize; ~0.3% of total FLOPs), gathers
each expert's tokens into a fixed-capacity transposed buffer, and the device
kernel runs the sparse SwiGLU FFN (gate/up/down matmuls) in bf16 with f32
PSUM accumulation. Outputs are scaled by the routing weight on-device and
scatter-added on the host.
"""

import numpy as np
from contextlib import ExitStack

import concourse.bass as bass
import concourse.tile as tile
from concourse import mybir
from concourse.bass_utils import run_bass_kernel_spmd

B, T, D, F, E, TOPK = 2, 1024, 2048, 768, 16, 2
NCORES = 8
EPC = E // NCORES  # experts per core
P = 128


def _split_waits(nc, max_waits=1):
    """walrus on this image rejects >1 sync-wait per instruction
    (setupSyncWait: "Too many sync wait commands"); split extras into
    preceding same-engine NoOps."""
    for f in nc.m.functions:
        for b in f.blocks:
            insts = b.instructions
            idx = 0
            while idx < len(insts):
                inst = insts[idx]
                si = getattr(inst, "sync_info", None)
                if si is not None and si.on_wait and len(si.on_wait) > max_waits:
                    waits = list(si.on_wait)
                    extra, keep = waits[:-max_waits], waits[-max_waits:]
                    pos = idx
                    for j in range(0, len(extra), max_waits):
                        chunk = extra[j : j + max_waits]
                        nop = mybir.InstNoOp(name=f"{inst.name}_ws{j}", ins=[], outs=[])
                        nop.engine = inst.engine
                        nop.sync_info = mybir.SyncInfo(on_wait=chunk, on_update=[])
                        insts.insert(pos, nop)
                        pos += 1
                        idx += 1
                    inst.sync_info = mybir.SyncInfo(
                        on_wait=keep, on_update=list(si.on_update)
                    )
                idx += 1


def build_moe(C):
    """Per-core kernel: EPC experts, capacity C tokens each (C % 128 == 0)."""
    KD = D // P  # 16 k-tiles over D
    KF = F // P  # 6 k-tiles over F
    KC = C // P  # token chunks
    ND = D // 512  # 4 n-tiles over D (down proj output)
    bf16 = mybir.dt.bfloat16
    f32 = mybir.dt.float32

    nc = bass.Bass("TRN2", target_bir_lowering=False, debug=False, num_devices=NCORES)
    xgt = nc.declare_dram_parameter("xgt", [EPC, D, C], bf16, isOutput=False)
    wg = nc.declare_dram_parameter("wg", [EPC, D, F], bf16, isOutput=False)
    wu = nc.declare_dram_parameter("wu", [EPC, D, F], bf16, isOutput=False)
    wd = nc.declare_dram_parameter("wd", [EPC, F, D], bf16, isOutput=False)
    cw = nc.declare_dram_parameter("cw", [EPC, C], f32, isOutput=False)
    y = nc.declare_dram_parameter("y", [EPC, C, D], f32, isOutput=True)

    with tile.TileContext(nc) as tc, ExitStack() as ctx:
        xp = ctx.enter_context(tc.tile_pool(name="xp", bufs=2))
        wgp = ctx.enter_context(tc.tile_pool(name="wgp", bufs=2))
        wup = ctx.enter_context(tc.tile_pool(name="wup", bufs=2))
        wdp = ctx.enter_context(tc.tile_pool(name="wdp", bufs=2))
        hp = ctx.enter_context(tc.tile_pool(name="hp", bufs=2))
        sp = ctx.enter_context(tc.tile_pool(name="sp", bufs=3))
        cp = ctx.enter_context(tc.tile_pool(name="cp", bufs=2))
        op = ctx.enter_context(tc.tile_pool(name="op", bufs=4))
        pg = ctx.enter_context(tc.tile_pool(name="pg", bufs=2, space="PSUM"))
        pu = ctx.enter_context(tc.tile_pool(name="pu", bufs=2, space="PSUM"))
        py = ctx.enter_context(tc.tile_pool(name="py", bufs=4, space="PSUM"))

        for e in range(EPC):
            # ---- loads (split into chunks so matmuls can start early) ----
            xt = xp.tile([P, KD, C], bf16, tag="xt")
            xsrc = xgt[e].rearrange("(k p) c -> p k c", p=P)
            for h in range(4):
                s = bass.ts(h, KD // 4)
                nc.sync.dma_start(xt[:, s, :], xsrc[:, s, :])

            gt = wgp.tile([P, KD, F], bf16, tag="gt")
            gsrc = wg[e].rearrange("(k p) f -> p k f", p=P)
            for h in range(4):
                s = bass.ts(h, KD // 4)
                nc.sync.dma_start(gt[:, s, :], gsrc[:, s, :])

            ut = wup.tile([P, KD, F], bf16, tag="ut")
            usrc = wu[e].rearrange("(k p) f -> p k f", p=P)
            for h in range(4):
                s = bass.ts(h, KD // 4)
                nc.sync.dma_start(ut[:, s, :], usrc[:, s, :])

            dt = wdp.tile([P, KF, D], bf16, tag="dt")
            dsrc = wd[e].rearrange("(k p) d -> p k d", p=P)
            for h in range(2):
                s = bass.ts(h, KF // 2)
                nc.sync.dma_start(dt[:, s, :], dsrc[:, s, :])

            ct = cp.tile([P, KC], f32, tag="ct")
            nc.sync.dma_start(ct[:], cw[e].rearrange("(k p) -> p k", p=P))

            # ---- gate/up + SwiGLU, producing hT [F, C] in bf16 ----
            ht = hp.tile([P, KF, C], bf16, tag="ht")
            for j in range(KF):
                g_ps = pg.tile([P, C], f32, tag="g_ps")
                u_ps = pu.tile([P, C], f32, tag="u_ps")
                for k in range(KD):
                    nc.tensor.matmul(
                        g_ps[:],
                        gt[:, k, bass.ts(j, P)],
                        xt[:, k, :],
                        start=(k == 0),
                        stop=(k == KD - 1),
                    )
                for k in range(KD):
                    nc.tensor.matmul(
                        u_ps[:],
                        ut[:, k, bass.ts(j, P)],
                        xt[:, k, :],
                        start=(k == 0),
                        stop=(k == KD - 1),
                    )
                sil = sp.tile([P, C], f32, tag="sil")
                nc.scalar.activation(
                    sil[:], g_ps[:], mybir.ActivationFunctionType.Silu
                )
                nc.vector.tensor_mul(ht[:, j, :], sil[:], u_ps[:])

            # ---- down proj + per-token routing-weight scale ----
            for kc in range(KC):
                for n in range(ND):
                    y_ps = py.tile([P, 512], f32, tag="y_ps")
                    for j in range(KF):
                        nc.tensor.matmul(
                            y_ps[:],
                            ht[:, j, bass.ts(kc, P)],
                            dt[:, j, bass.ts(n, 512)],
                            start=(j == 0),
                            stop=(j == KF - 1),
                        )
                    yt = op.tile([P, 512], f32, tag="yt")
                    nc.vector.tensor_scalar_mul(yt[:], y_ps[:], ct[:, kc : kc + 1])
                    nc.sync.dma_start(
                        y[e, bass.ts(kc, P), bass.ts(n, 512)], yt[:]
                    )

    _split_waits(nc)
    return nc


_CACHE = {}


def _get_nc(C):
    if C not in _CACHE:
        _CACHE[C] = build_moe(C)
    return _CACHE[C]


def _route(x, router_w):
    """Replicates the reference router in f32: softmax over expert scores,
    top-2, renormalize."""
    xf = x.reshape(-1, D).astype(np.float32)
    scores = xf @ router_w.astype(np.float32)
    m = scores.max(axis=-1, keepdims=True)
    ex = np.exp(scores - m)
    probs = ex / ex.sum(axis=-1, keepdims=True)
    idx = np.argsort(-probs, axis=-1, kind="stable")[:, :TOPK]
    wts = np.take_along_axis(probs, idx, axis=-1)
    wts = wts / wts.sum(axis=-1, keepdims=True)
    return idx.astype(np.int32), wts.astype(np.float32)


def kernel(x, router_w, gate_w, up_w, down_w):
    x = np.asarray(x)
    in_dtype = x.dtype
    xf = x.reshape(-1, D).astype(np.float32)
    idx, wts = _route(x, np.asarray(router_w))

    # token lists per expert
    tok_ids = [None] * E
    tok_wts = [None] * E
    for e in range(E):
        sel = np.nonzero(idx == e)
        tok_ids[e] = sel[0].astype(np.int64)
        tok_wts[e] = wts[sel[0], sel[1]]
    max_n = max(len(t) for t in tok_ids)
    C = max(P, -(-max_n // P) * P)

    nc = _get_nc(C)

    import ml_dtypes

    bf = ml_dtypes.bfloat16
    g16 = np.asarray(gate_w).astype(bf)
    u16 = np.asarray(up_w).astype(bf)
    d16 = np.asarray(down_w).astype(bf)
    xT = np.ascontiguousarray(xf.T).astype(bf)  # [D, B*T]

    in_maps = []
    for c in range(NCORES):
        xg = np.zeros((EPC, D, C), dtype=bf)
        cwv = np.zeros((EPC, C), dtype=np.float32)
        for j in range(EPC):
            e = c * EPC + j
            n = len(tok_ids[e])
            xg[j, :, :n] = xT[:, tok_ids[e]]
            cwv[j, :n] = tok_wts[e]
        in_maps.append(
            {
                "xgt": xg,
                "wg": np.ascontiguousarray(g16[c * EPC : (c + 1) * EPC]),
                "wu": np.ascontiguousarray(u16[c * EPC : (c + 1) * EPC]),
                "wd": np.ascontiguousarray(d16[c * EPC : (c + 1) * EPC]),
                "cw": cwv,
            }
        )

    res = run_bass_kernel_spmd(nc, in_maps, list(range(NCORES)))

    out = np.zeros((B * T, D), dtype=np.float32)
    for c in range(NCORES):
        yv = res.results[c]["y"]
        for j in range(EPC):
            e = c * EPC + j
            n = len(tok_ids[e])
            out[tok_ids[e]] += yv[j, :n, :].astype(np.float32)
    return out.reshape(B, T, D).astype(in_dtype)


# revision 7
# speedup vs baseline: 1.0375x; 1.0375x over previous
"""MoE layer (B=2,T=1024,D=2048,F=768,E=16,K=2) on 8 NeuronCores.

Expert-parallel: core c owns experts {2c, 2c+1}. Host computes the router
(scores -> softmax -> top-2 -> renormalize; ~0.3% of total FLOPs), gathers
each expert's tokens into a fixed-capacity transposed buffer, and the device
kernel runs the sparse SwiGLU FFN (gate/up/down matmuls) in bf16 with f32
PSUM accumulation. Outputs are scaled by the routing weight on-device and
scatter-added on the host.

Layout: tokens are staged transposed (xgt [D, C]) so gate/up matmuls produce
hT [F, C] directly in the lhsT layout the down projection needs -- no
on-device transposes anywhere.
"""

import numpy as np
from contextlib import ExitStack

import concourse.bass as bass
import concourse.tile as tile
from concourse import mybir
from concourse.bass_utils import run_bass_kernel_spmd

B, T, D, F, E, TOPK = 2, 1024, 2048, 768, 16, 2
NCORES = 8
EPC = E // NCORES  # experts per core
P = 128


def _split_waits(nc, max_waits=1):
    """walrus on this image rejects >1 sync-wait per instruction
    (setupSyncWait: "Too many sync wait commands"); split extras into
    preceding same-engine NoOps."""
    for f in nc.m.functions:
        for b in f.blocks:
            insts = b.instructions
            idx = 0
            while idx < len(insts):
                inst = insts[idx]
                si = getattr(inst, "sync_info", None)
                if si is not None and si.on_wait and len(si.on_wait) > max_waits:
                    waits = list(si.on_wait)
                    extra, keep = waits[:-max_waits], waits[-max_waits:]
                    pos = idx
                    for j in range(0, len(extra), max_waits):
                        chunk = extra[j : j + max_waits]
                        nop = mybir.InstNoOp(name=f"{inst.name}_ws{j}", ins=[], outs=[])
                        nop.engine = inst.engine
                        nop.sync_info = mybir.SyncInfo(on_wait=chunk, on_update=[])
                        insts.insert(pos, nop)
                        pos += 1
                        idx += 1
                    inst.sync_info = mybir.SyncInfo(
                        on_wait=keep, on_update=list(si.on_update)
                    )
                idx += 1


def build_moe(C):
    """Per-core kernel: EPC experts, capacity C tokens each (C % 64 == 0)."""
    assert C % 64 == 0 and C <= 512
    KD = D // P  # 16 k-tiles over D
    KF = F // P  # 6 f-chunks over F
    ND = D // 512  # 4 n-tiles over D (down proj output)
    XS = 4  # xt DMA slabs
    # token chunks for the down projection (partition dim), e.g. 320 -> 128,128,64
    chunks = []
    off = 0
    while off < C:
        chunks.append((off, min(P, C - off)))
        off += P
    bf16 = mybir.dt.bfloat16
    f32 = mybir.dt.float32

    nc = bass.Bass("TRN2", target_bir_lowering=False, debug=False, num_devices=NCORES)
    xgt = nc.declare_dram_parameter("xgt", [EPC, D, C], bf16, isOutput=False)
    wg = nc.declare_dram_parameter("wg", [EPC, D, F], bf16, isOutput=False)
    wu = nc.declare_dram_parameter("wu", [EPC, D, F], bf16, isOutput=False)
    wd = nc.declare_dram_parameter("wd", [EPC, F, D], bf16, isOutput=False)
    CP = len(chunks) * P  # cw padded to a multiple of 128 for the rearrange
    cw = nc.declare_dram_parameter("cw", [EPC, CP], f32, isOutput=False)
    y = nc.declare_dram_parameter("y", [EPC, C, D], bf16, isOutput=True)

    with tile.TileContext(nc) as tc, ExitStack() as ctx:
        xp = ctx.enter_context(tc.tile_pool(name="xp", bufs=2))
        wgp = ctx.enter_context(tc.tile_pool(name="wgp", bufs=2))
        wup = ctx.enter_context(tc.tile_pool(name="wup", bufs=2))
        wdp = ctx.enter_context(tc.tile_pool(name="wdp", bufs=2))
        hp = ctx.enter_context(tc.tile_pool(name="hp", bufs=2))
        sp = ctx.enter_context(tc.tile_pool(name="sp", bufs=3))
        cp = ctx.enter_context(tc.tile_pool(name="cp", bufs=2))
        op = ctx.enter_context(tc.tile_pool(name="op", bufs=4))
        pg = ctx.enter_context(tc.tile_pool(name="pg", bufs=2, space="PSUM"))
        pu = ctx.enter_context(tc.tile_pool(name="pu", bufs=2, space="PSUM"))
        py = ctx.enter_context(tc.tile_pool(name="py", bufs=4, space="PSUM"))

        for e in range(EPC):
            # ---- loads. gate/up are loaded as per-f-chunk column slabs so the
            # j-th matmul chain only depends on slab j (the j=0 chain can start
            # ~5us in instead of waiting for the whole 6MB of gate+up).
            ct = cp.tile([P, len(chunks)], f32, tag="ct")
            nc.gpsimd.dma_start(ct[:], cw[e].rearrange("(k p) -> p k", p=P))

            xts = []
            xsrc = xgt[e].rearrange("(k p) c -> p k c", p=P)
            for h in range(XS):
                xt = xp.tile([P, KD // XS, C], bf16, tag=f"xt{h}")
                nc.gpsimd.dma_start(xt[:], xsrc[:, bass.ts(h, KD // XS), :])
                xts.append(xt)

            gsrc = wg[e].rearrange("(k p) f -> p k f", p=P)
            usrc = wu[e].rearrange("(k p) f -> p k f", p=P)
            gts, uts = [], []
            for j in range(KF):
                gt = wgp.tile([P, KD, P], bf16, tag=f"gt{j}")
                nc.sync.dma_start(gt[:], gsrc[:, :, bass.ts(j, P)])
                gts.append(gt)
                ut = wup.tile([P, KD, P], bf16, tag=f"ut{j}")
                nc.sync.dma_start(ut[:], usrc[:, :, bass.ts(j, P)])
                uts.append(ut)

            dsrc = wd[e].rearrange("(k p) d -> p k d", p=P)
            dts = []
            for h in range(2):
                dt = wdp.tile([P, KF // 2, D], bf16, tag=f"dt{h}")
                nc.sync.dma_start(dt[:], dsrc[:, bass.ts(h, KF // 2), :])
                dts.append(dt)

            # ---- gate/up + SwiGLU, producing hT [F, C] in bf16 ----
            ht = hp.tile([P, KF, C], bf16, tag="ht")
            for j in range(KF):
                g_ps = pg.tile([P, C], f32, tag="g_ps")
                u_ps = pu.tile([P, C], f32, tag="u_ps")
                for k in range(KD):
                    nc.tensor.matmul(
                        g_ps[:],
                        gts[j][:, k, :],
                        xts[k // (KD // XS)][:, k % (KD // XS), :],
                        start=(k == 0),
                        stop=(k == KD - 1),
                    )
                for k in range(KD):
                    nc.tensor.matmul(
                        u_ps[:],
                        uts[j][:, k, :],
                        xts[k // (KD // XS)][:, k % (KD // XS), :],
                        start=(k == 0),
                        stop=(k == KD - 1),
                    )
                sil = sp.tile([P, C], f32, tag="sil")
                nc.scalar.activation(
                    sil[:], g_ps[:], mybir.ActivationFunctionType.Silu
                )
                nc.vector.tensor_mul(ht[:, j, :], sil[:], u_ps[:])

            # ---- down proj + per-token routing-weight scale ----
            for kc, (coff, clen) in enumerate(chunks):
                for n in range(ND):
                    y_ps = py.tile([P, 512], f32, tag="y_ps")
                    for j in range(KF):
                        nc.tensor.matmul(
                            y_ps[:clen, :],
                            ht[:, j, coff : coff + clen],
                            dts[j // (KF // 2)][:, j % (KF // 2), bass.ts(n, 512)],
                            start=(j == 0),
                            stop=(j == KF - 1),
                        )
                    yt = op.tile([P, 512], bf16, tag="yt")
                    nc.vector.tensor_scalar_mul(
                        yt[:clen, :], y_ps[:clen, :], ct[:clen, kc : kc + 1]
                    )
                    nc.gpsimd.dma_start(
                        y[e, coff : coff + clen, bass.ts(n, 512)], yt[:clen, :]
                    )

    _split_waits(nc)
    return nc


_CACHE = {}


def _get_nc(C):
    if C not in _CACHE:
        _CACHE[C] = build_moe(C)
    return _CACHE[C]


def _route(x, router_w):
    """Replicates the reference router in f32: softmax over expert scores,
    top-2, renormalize."""
    xf = x.reshape(-1, D).astype(np.float32)
    scores = xf @ router_w.astype(np.float32)
    m = scores.max(axis=-1, keepdims=True)
    ex = np.exp(scores - m)
    probs = ex / ex.sum(axis=-1, keepdims=True)
    idx = np.argsort(-probs, axis=-1, kind="stable")[:, :TOPK]
    wts = np.take_along_axis(probs, idx, axis=-1)
    wts = wts / wts.sum(axis=-1, keepdims=True)
    return idx.astype(np.int32), wts.astype(np.float32)


def kernel(x, router_w, gate_w, up_w, down_w):
    import ml_dtypes

    bf = ml_dtypes.bfloat16

    x = np.asarray(x)
    in_dtype = x.dtype
    xf = x.reshape(-1, D).astype(np.float32)
    idx, wts = _route(x, np.asarray(router_w))

    # token lists per expert
    tok_ids = [None] * E
    tok_wts = [None] * E
    for e in range(E):
        sel = np.nonzero(idx == e)
        tok_ids[e] = sel[0].astype(np.int64)
        tok_wts[e] = wts[sel[0], sel[1]]
    max_n = max(len(t) for t in tok_ids)
    C = min(512, max(P, -(-max_n // 64) * 64))

    nc = _get_nc(C)

    g16 = np.asarray(gate_w).astype(bf)
    u16 = np.asarray(up_w).astype(bf)
    d16 = np.asarray(down_w).astype(bf)
    xT = np.ascontiguousarray(xf.T).astype(bf)  # [D, B*T]

    in_maps = []
    for c in range(NCORES):
        xg = np.zeros((EPC, D, C), dtype=bf)
        cwv = np.zeros((EPC, -(-C // P) * P), dtype=np.float32)
        for j in range(EPC):
            e = c * EPC + j
            n = len(tok_ids[e])
            xg[j, :, :n] = xT[:, tok_ids[e]]
            cwv[j, :n] = tok_wts[e]
        in_maps.append(
            {
                "xgt": xg,
                "wg": np.ascontiguousarray(g16[c * EPC : (c + 1) * EPC]),
                "wu": np.ascontiguousarray(u16[c * EPC : (c + 1) * EPC]),
                "wd": np.ascontiguousarray(d16[c * EPC : (c + 1) * EPC]),
                "cw": cwv,
            }
        )

    res = run_bass_kernel_spmd(nc, in_maps, list(range(NCORES)))

    out = np.zeros((B * T, D), dtype=np.float32)
    for c in range(NCORES):
        yv = res.results[c]["y"]
        for j in range(EPC):
            e = c * EPC + j
            n = len(tok_ids[e])
            out[tok_ids[e]] += yv[j, :n, :].astype(np.float32)
    return out.reshape(B, T, D).astype(in_dtype)
